# revision 20
# baseline (speedup 1.0000x reference)
"""Causal self-attention (B=2, T=2048, C=1024, H=16, D=64) on 8 NeuronCores.

Sharding: core = (batch b, head-group g); each of the 8 cores handles one
batch and 4 of the 16 heads (data parallel on B, tensor parallel on heads).
Each core computes q/k/v projections for its heads, rope, causal softmax
attention, and a partial out-projection; the host sums the 4 per-batch
partials and adds bout.

Device dataflow (per core), all matmuls in fp32r (full PE rate, ~1.5e-4 rel):
  - host passes x[b].T so the contraction dim (C) lands on partitions
  - q,k are produced directly in [dim, t] layout ("A"=low rotary halves of
    4 heads stacked, "B"=high halves), rope applied with DVE/GpSimd
  - S^T = K~^T Q~ per 128k x 512q block, 4 heads packed into the PE array
    via tile_position row groups (contraction=32 each for A/B parts)
  - softmax without max-subtraction (logits are O(1) for this model):
    exp on ScalarE with the 1/8 scale folded in; causal masking by
    multiplying diagonal blocks with constant 0/1 masks on GpSimd
  - O^T = V_aug^T expS^T accumulated over k blocks, where V_aug carries a
    ones column so row 64 of the PSUM accumulator is the softmax
    denominator; divide via DVE reciprocal + K=1 PE broadcast of the recip
  - partial out-projection [t,c] = (O^T)^T Wout_rows, DMA'd out naturally
"""
import sys
sys.path.insert(0, '/opt/trn_rl_repo')

import numpy as np
from contextlib import ExitStack

import concourse.bass as bass
import concourse.tile as tile
from concourse import mybir
from concourse.bass_utils import run_bass_kernel_spmd

B, T, C, H, D = 2, 2048, 1024, 16, 64
HPC = 4          # heads per core
G = H // HPC     # head groups (cores per batch)
N_CORES = B * G
SCALE = 1.0 / np.sqrt(D)
P = 128
QT = 512         # q tile width
TT = T // QT     # q tiles
NKB = T // P     # 128-wide k blocks
NTB = T // P     # 128-wide t blocks
NCC = C // P     # 128-deep contraction chunks
F32 = mybir.dt.float32
F32R = mybir.dt.float32r


def _tril_mask():
    p = np.arange(P)[:, None]
    f = np.arange(P)[None, :]
    return (p <= f).astype(np.float32)


# walrus in this toolchain can't encode >1 sem wait on one instruction
# ("Too many sync wait commands"); split excess waits onto preceding NoOps.
def _split_waits(nc, maxw=1):
    for f in nc.m.functions:
        for bb in f.blocks:
            out = []
            for inst in bb.instructions:
                si = getattr(inst, 'sync_info', None)
                if si is not None and si.on_wait and len(si.on_wait) > maxw:
                    waits = list(si.on_wait)
                    extra, keep = waits[:-maxw], waits[-maxw:]
                    for i in range(0, len(extra), maxw):
                        out.append(mybir.InstNoOp(
                            name=f"{inst.name}-wsplit{i}",
                            sync_info=mybir.SyncInfo(
                                on_wait=extra[i:i + maxw], on_update=[]),
                            bass_nofuse=True,
                            engine=inst.engine,
                        ))
                    inst.sync_info = mybir.SyncInfo(
                        on_wait=keep, on_update=list(si.on_update or []))
                out.append(inst)
            bb.instructions[:] = out


def build_nc(split=True):
    nc = bass.Bass()
    xT = nc.dram_tensor("xT", [C, T], F32R, kind="ExternalInput")
    wq = nc.dram_tensor("wq", [C, 256], F32R, kind="ExternalInput")
    wk = nc.dram_tensor("wk", [C, 256], F32R, kind="ExternalInput")
    wv = nc.dram_tensor("wv", [C, 260], F32R, kind="ExternalInput")
    bqk = nc.dram_tensor("bqk", [P, 4], F32, kind="ExternalInput")  # qA qB kA kB
    bv = nc.dram_tensor("bv", [P, 2], F32, kind="ExternalInput")
    scs = nc.dram_tensor("scs", [P, 2 * T], F32, kind="ExternalInput")  # sin|cos
    wout = nc.dram_tensor("wout", [256, C], F32R, kind="ExternalInput")
    y = nc.dram_tensor("y", [T, C], F32, kind="ExternalOutput")
    masks_d = nc.inline_tensor(_tril_mask(), name="cmasks")

    with tile.TileContext(nc) as tc:
        with ExitStack() as ctx:
            # ---- resident pools ----
            wpool = ctx.enter_context(tc.tile_pool(name="wts", bufs=1))
            qkpool = ctx.enter_context(tc.tile_pool(name="qk", bufs=1))
            vpool = ctx.enter_context(tc.tile_pool(name="v", bufs=1))
            otpool = ctx.enter_context(tc.tile_pool(name="ot", bufs=1))

            bqk_sb = wpool.tile([P, 4], F32, tag="bqk")
            nc.sync.dma_start(bqk_sb[:], bqk[:])
            bv_sb = wpool.tile([P, 2], F32, tag="bv")
            nc.sync.dma_start(bv_sb[:], bv[:])
            wq_sb = wpool.tile([P, NCC, 256], F32R, tag="wq")
            nc.sync.dma_start(wq_sb[:], wq.rearrange("(o p) n -> p o n", p=P))
            wk_sb = wpool.tile([P, NCC, 256], F32R, tag="wk")
            wv_sb = wpool.tile([P, NCC, 260], F32R, tag="wv")
            scs_sb = wpool.tile([P, 2 * T], F32, tag="scs")
            sin4 = scs_sb[:, 0:T]
            cos4 = scs_sb[:, T:2 * T]
            wout_sb = wpool.tile([P, 2, C], F32R, tag="wout")
            masks_sb = wpool.tile([P, P], F32R, tag="masks")
            ones_f = wpool.tile([P, 64], F32, tag="onesf")
            nc.vector.memset(ones_f[:], 1.0)
            ones_sb = wpool.tile([P, 64], F32R, tag="ones")
            nc.vector.tensor_copy(ones_sb[:], ones_f[:])

            # q/k in rotary-half layout: A = low halves of 4 heads, B = high
            qA = qkpool.tile([P, T], F32R, tag="qA")
            qB = qkpool.tile([P, T], F32R, tag="qB")
            kA = qkpool.tile([P, T], F32R, tag="kA")
            kB = qkpool.tile([P, T], F32R, tag="kB")
            qk_tiles = [qA, qB, kA, kB]
            w_of = {0: wq_sb, 1: wq_sb, 2: wk_sb, 3: wk_sb}
            col_of = {0: 0, 1: 128, 2: 0, 3: 128}

            # V tiles [t-block, 4*65] (65th col per head becomes ones)
            v_tiles = [vpool.tile([P, 260], F32R, tag=f"v{tb}", name=f"v{tb}")
                       for tb in range(NTB)]

            # O^T: heads 0,1 stacked / heads 2,3 stacked
            ot_sb = [otpool.tile([P, T], F32R, tag=f"otsb{i}", name=f"otsb{i}")
                     for i in range(2)]

            with ExitStack() as stream:
                xpool = stream.enter_context(tc.tile_pool(name="x", bufs=2))
                rtmp = stream.enter_context(tc.tile_pool(name="rtmp", bufs=4))
                ps_s = stream.enter_context(
                    tc.tile_pool(name="pss", bufs=2, space="PSUM"))
                ps_ot = stream.enter_context(
                    tc.tile_pool(name="psot", bufs=2, space="PSUM"))
                espool = stream.enter_context(tc.tile_pool(name="es", bufs=5))
                dpool = stream.enter_context(tc.tile_pool(name="dv", bufs=2))
                opool = stream.enter_context(tc.tile_pool(name="osb", bufs=2))
                qtmp = stream.enter_context(tc.tile_pool(name="qtmp", bufs=2))

                xt = {}

                def load_xt(tt):
                    for cc in range(NCC):
                        t = xpool.tile([P, QT], F32R, tag=f"x{cc}",
                                       name=f"x{cc}_{tt}")
                        nc.sync.dma_start(
                            t[:], xT[cc * P:(cc + 1) * P,
                                     tt * QT:(tt + 1) * QT])
                        xt[(cc, tt)] = t

                # q/k projection for one (jb, tt): contraction split in two
                # 4-chunk PSUM generations so a pss slot is never held long.
                def qkproj(jb, tt):
                    wsb, c0 = w_of[jb], col_of[jb]
                    dst = qk_tiles[jb][:, tt * QT:(tt + 1) * QT]
                    psa = ps_s.tile([P, 2, QT], F32, tag="pss",
                                    name="qk_a")[:, 0, :]
                    for cc in range(4):
                        nc.tensor.matmul(
                            psa, wsb[:, cc, c0:c0 + 128], xt[(cc, tt)][:],
                            start=(cc == 0), stop=(cc == 3))
                    half = qtmp.tile([P, QT], F32, tag="qh", name="qh")
                    nc.scalar.activation(
                        half[:], psa, mybir.ActivationFunctionType.Identity,
                        bias=bqk_sb[:, jb:jb + 1], scale=1.0)
                    psb = ps_s.tile([P, 2, QT], F32, tag="pss",
                                    name="qk_b")[:, 0, :]
                    for cc in range(4, NCC):
                        nc.tensor.matmul(
                            psb, wsb[:, cc, c0:c0 + 128], xt[(cc, tt)][:],
                            start=(cc == 4), stop=(cc == NCC - 1))
                    nc.vector.tensor_add(dst, psb, half[:])

                def rope(pair, tt):
                    At, Bt = qk_tiles[2 * pair], qk_tiles[2 * pair + 1]
                    s = slice(tt * QT, (tt + 1) * QT)
                    t1 = rtmp.tile([P, QT], F32R, tag="rt", name="rt1")
                    t2 = rtmp.tile([P, QT], F32R, tag="rt", name="rt2")
                    t3 = rtmp.tile([P, QT], F32R, tag="rt", name="rt3")
                    cosr = cos4[:, s].bitcast(F32R)
                    sinr = sin4[:, s].bitcast(F32R)
                    nc.gpsimd.tensor_mul(t1[:], At[:, s], cosr)
                    nc.gpsimd.tensor_mul(t2[:], Bt[:, s], sinr)
                    nc.gpsimd.tensor_mul(t3[:], At[:, s], sinr)
                    nc.vector.tensor_sub(At[:, s], t1[:], t2[:])
                    nc.vector.tensor_mul(Bt[:, s], Bt[:, s], cosr)
                    nc.vector.tensor_add(Bt[:, s], Bt[:, s], t3[:])

                def vproj(tb):
                    ps = ps_s.tile([P, 2, QT], F32, tag="pss",
                                   name="psv")[:, 0, 0:260]
                    for cc in range(NCC):
                        nc.tensor.matmul(
                            ps, xt[(cc, tb // 4)][:, (tb % 4) * P:
                                                  (tb % 4 + 1) * P],
                            wv_sb[:, cc, :],
                            start=(cc == 0), stop=(cc == NCC - 1))
                    nc.vector.tensor_copy(v_tiles[tb][:], ps)
                    ones_cols = v_tiles[tb].rearrange(
                        "p (h e) -> p h e", e=65)[:, :, 64]
                    nc.vector.tensor_scalar_add(ones_cols, ones_cols, 1.0)

                def divides_a(tt, ot2):
                    pend = []
                    for pp in range(2):
                        otf2 = dpool.tile([65, 2, QT], F32R, tag="otf",
                                          name="otf")
                        nc.scalar.activation(
                            otf2[:], ot2[pp][:],
                            mybir.ActivationFunctionType.Copy)
                        dn = dpool.tile([P, 8], F32, tag="dn", name="dn")
                        nc.sync.dma_start(
                            dn[:], otf2[64:65, :, :].rearrange(
                                "a b c -> a (b c)").bitcast(F32))
                        nc.vector.reciprocal(dn[:], dn[:])
                        rr = dpool.tile([1, 2, QT], F32R, tag="rr", name="rr")
                        nc.sync.dma_start(
                            rr[0:1, :, :].rearrange("a b c -> a (b c)"),
                            dn[:].bitcast(F32R))
                        pend.append((otf2, rr))
                    return pend

                def divides_b(tt, pend):
                    for pp in range(2):
                        otf2, rr = pend[pp]
                        for j in range(2):
                            rb = ps_s.tile([P, 2, QT], F32, tag="pss",
                                           name="rb")[0:64, 0, :]
                            nc.tensor.matmul(
                                rb, ones_sb[0:1, 0:64],
                                rr[0:1, j, :], start=True, stop=True)
                            rbs = dpool.tile([64, QT], F32R, tag="rbs",
                                             name="rbs")
                            nc.vector.tensor_copy(rbs[:], rb.bitcast(F32R))
                            dst = ot_sb[pp][64 * j:64 * j + 64,
                                            tt * QT:(tt + 1) * QT]
                            nc.vector.tensor_mul(dst, otf2[0:64, j, :], rbs[:])
                            nc.vector.tensor_scalar_add(
                                dst, dst, bv_sb[64 * j:64 * j + 64, pp:pp + 1])

                def outproj(tb):
                    o_sb = opool.tile([P, C], F32, tag="osb", name="osb")
                    for nt in range(2):
                        ps = ps_s.tile([P, 2, QT], F32, tag="pss",
                                       name="pso")[:, 0, :]
                        for rc in range(2):
                            nc.tensor.matmul(
                                ps, ot_sb[rc][:, tb * P:(tb + 1) * P],
                                wout_sb[:, rc, nt * 512:(nt + 1) * 512],
                                start=(rc == 0), stop=(rc == 1))
                        nc.vector.tensor_copy(
                            o_sb[:, nt * 512:(nt + 1) * 512], ps)
                    nc.sync.dma_start(y[tb * P:(tb + 1) * P, :], o_sb[:])

                # ---- prologue: tile 0's inputs and projections ----
                load_xt(0)
                nc.sync.dma_start(wk_sb[:], wk.rearrange("(o p) n -> p o n", p=P))
                nc.sync.dma_start(scs_sb[:], scs[:])
                nc.sync.dma_start(wv_sb[:], wv.rearrange("(o p) n -> p o n", p=P))
                nc.sync.dma_start(masks_sb[:], masks_d[:].bitcast(F32R))
                nc.sync.dma_start(wout_sb[:],
                                  wout.rearrange("(o p) n -> p o n", p=P))
                for jb in range(4):
                    qkproj(jb, 0)
                rope(0, 0)
                rope(1, 0)
                for tb in range(4):
                    vproj(tb)

                # ---- streaming attention with injected work ----
                queue = []  # closures of next-tile + prev-tile work
                prev = None
                for tt in range(TT):
                    nk = 4 * tt + 4
                    if tt + 1 < TT:
                        ntt = tt + 1
                        load_xt(ntt)
                        queue += [lambda jb=jb, t=ntt: qkproj(jb, t)
                                  for jb in range(4)]
                        queue += [lambda t=ntt: rope(0, t),
                                  lambda t=ntt: rope(1, t)]
                        queue += [lambda tb=tb: vproj(tb)
                                  for tb in range(4 * ntt, 4 * ntt + 4)]
                    ot2 = [ps_ot.tile([65, 2, QT], F32, tag="psot",
                                      name=f"psot{pp}") for pp in range(2)]
                    es_prev = [None, None]
                    off_prev = [0, 0]
                    for kblk in range(nk):
                        off = max(0, (kblk - 4 * tt)) * P
                        ks = slice(kblk * P, (kblk + 1) * P)
                        qs = slice(tt * QT + off, (tt + 1) * QT)
                        for pp in range(2):
                            s2 = ps_s.tile([P, 2, QT], F32, tag="pss",
                                           name="pss")
                            for j in range(2):
                                h = 2 * pp + j
                                hp = slice(32 * h, 32 * h + 32)
                                nc.tensor.matmul(
                                    s2[:, j, off:], kA[hp, ks], qA[hp, qs],
                                    start=True, stop=False,
                                    tile_position=(32 * h, 0))
                                nc.tensor.matmul(
                                    s2[:, j, off:], kB[hp, ks], qB[hp, qs],
                                    start=False, stop=True,
                                    tile_position=(32 * h, 0))
                            es2 = espool.tile([P, 2, QT], F32R, tag="es",
                                              name="es")
                            nc.scalar.activation(
                                es2[:, :, off:], s2[:, :, off:],
                                mybir.ActivationFunctionType.Exp, scale=SCALE)
                            if kblk >= 4 * tt:
                                eng = nc.vector if kblk % 2 == 0 else nc.gpsimd
                                eng.tensor_mul(
                                    es2[:, :, off:off + P],
                                    es2[:, :, off:off + P],
                                    masks_sb[:, None, :].to_broadcast(
                                        (P, 2, P)))
                            if kblk > 0:
                                for j in range(2):
                                    h = 2 * pp + j
                                    nc.tensor.matmul(
                                        ot2[pp][:, j, off_prev[pp]:],
                                        v_tiles[kblk - 1][:, 65 * h:65 * h + 65],
                                        es_prev[pp][:, j, off_prev[pp]:],
                                        start=(kblk == 1), stop=False)
                            es_prev[pp], off_prev[pp] = es2, off
                        # injected pipeline work
                        if prev is not None:
                            ptt, pend = prev
                            if kblk == 2:
                                divides_b(ptt, pend)
                            elif 3 <= kblk <= 6:
                                outproj(4 * ptt + kblk - 3)
                                if kblk == 6:
                                    prev = None
                        rounds_left = nk - 1 - kblk
                        if queue:
                            npop = max(1, -(-len(queue) // max(1, rounds_left)))                                 if rounds_left > 0 else len(queue)
                            for _ in range(min(npop, len(queue))):
                                queue.pop(0)()
                    for pp in range(2):
                        for j in range(2):
                            h = 2 * pp + j
                            nc.tensor.matmul(
                                ot2[pp][:, j, off_prev[pp]:],
                                v_tiles[nk - 1][:, 65 * h:65 * h + 65],
                                es_prev[pp][:, j, off_prev[pp]:],
                                start=(nk == 1), stop=True)
                    prev = (tt, divides_a(tt, ot2))
                # tail
                ptt, pend = prev
                divides_b(ptt, pend)
                for tb in range(4 * ptt, 4 * ptt + 4):
                    outproj(tb)

    if split:
        _split_waits(nc)
    return nc


def make_in_maps(x, rope_cache, Wqkv, bqkv, Wout, bout):
    """Host-side shard prep. Returns list of 8 in_maps (core = 4*b + g)."""
    x = np.asarray(x, np.float32)
    rope_cache = np.asarray(rope_cache, np.float32)
    Wqkv = np.asarray(Wqkv, np.float32)
    bqkv = np.asarray(bqkv, np.float32)
    Wout = np.asarray(Wout, np.float32)

    # rotary-half permutation within a head: [evens, odds]
    perm = np.concatenate([np.arange(0, D, 2), np.arange(1, D, 2)])
    sin = rope_cache[:, 0::2].T.copy()   # [32, T]
    cos = rope_cache[:, 1::2].T.copy()
    scs = np.concatenate([np.tile(sin, (4, 1)), np.tile(cos, (4, 1))],
                         axis=1).astype(np.float32)  # [128, 2T]

    xT = [np.ascontiguousarray(x[b].T) for b in range(B)]

    in_maps = []
    for core in range(N_CORES):
        b, g = divmod(core, G)
        heads = range(HPC * g, HPC * g + HPC)
        # A-block: low halves (even dims) of the 4 heads; B-block: high halves
        qcols, kcols, vcols = [], [], []
        for part in range(2):  # lo, hi
            for h in heads:
                dd = h * D + perm[part * 32:(part + 1) * 32]
                qcols.extend(0 * C + dd)
                kcols.extend(1 * C + dd)
        for h in heads:
            vcols.extend(2 * C + h * D + np.arange(D))
        qcols = np.asarray(qcols)
        kcols = np.asarray(kcols)
        vcols = np.asarray(vcols)
        wq_c = np.ascontiguousarray(Wqkv[:, qcols])
        wk_c = np.ascontiguousarray(Wqkv[:, kcols])
        wv_c = np.zeros((C, 260), np.float32)
        vv = Wqkv[:, vcols]
        for h in range(HPC):
            wv_c[:, 65 * h:65 * h + 64] = vv[:, 64 * h:64 * h + 64]
        bqk_c = np.stack([bqkv[qcols[:128]], bqkv[qcols[128:]],
                          bqkv[kcols[:128]], bqkv[kcols[128:]]], axis=1)
        bv_c = bqkv[vcols].reshape(2, 128).T
        rows = np.arange(HPC * g * D, (HPC * g + HPC) * D)
        wout_c = np.ascontiguousarray(Wout[rows, :])
        in_maps.append({
            "xT": xT[b], "wq": wq_c, "wk": wk_c,
            "wv": np.ascontiguousarray(wv_c),
            "bqk": np.ascontiguousarray(bqk_c.astype(np.float32)),
            "bv": np.ascontiguousarray(bv_c.astype(np.float32)),
            "scs": scs, "wout": wout_c,
        })
    return in_maps


_NC_CACHE = None


def _get_nc():
    global _NC_CACHE
    if _NC_CACHE is None:
        _NC_CACHE = build_nc()
    return _NC_CACHE


def run(inputs, trace=False):
    nc = _get_nc()
    in_maps = make_in_maps(**inputs)
    res = run_bass_kernel_spmd(nc, in_maps, list(range(N_CORES)), trace=trace)
    bout = np.asarray(inputs["bout"], np.float32)
    out = np.zeros((B, T, C), np.float32)
    for core in range(N_CORES):
        out[core // G] += res.results[core]["y"]
    out += bout[None, None, :]
    return out, res


def kernel(**inputs):
    out, _ = run(inputs)
    return out


# revision 21
# speedup vs baseline: 1.0180x; 1.0180x over previous
"""Causal self-attention (B=2, T=2048, C=1024, H=16, D=64) on 8 NeuronCores.

Sharding: core = (batch b, head-group g); each of the 8 cores handles one
batch and 4 of the 16 heads (data parallel on B, tensor parallel on heads).
Each core computes q/k/v projections for its heads, rope, causal softmax
attention, and a partial out-projection; the host sums the 4 per-batch
partials and adds bout.

Device dataflow (per core), all matmuls in fp32r (full PE rate, ~1.5e-4 rel):
  - host passes x[b].T so the contraction dim (C) lands on partitions
  - q,k are produced directly in [dim, t] layout ("A"=low rotary halves of
    4 heads stacked, "B"=high halves), rope applied with DVE/GpSimd
  - S^T = K~^T Q~ per 128k x 512q block, 4 heads packed into the PE array
    via tile_position row groups (contraction=32 each for A/B parts)
  - softmax without max-subtraction (logits are O(1) for this model):
    exp on ScalarE with the 1/8 scale folded in; causal masking by
    multiplying diagonal blocks with constant 0/1 masks on GpSimd
  - O^T = V_aug^T expS^T accumulated over k blocks, where V_aug carries a
    ones column so row 64 of the PSUM accumulator is the softmax
    denominator; divide via DVE reciprocal + K=1 PE broadcast of the recip
  - partial out-projection [t,c] = (O^T)^T Wout_rows, DMA'd out naturally
"""
import sys
sys.path.insert(0, '/opt/trn_rl_repo')

import numpy as np
from contextlib import ExitStack

import concourse.bass as bass
import concourse.tile as tile
from concourse import mybir
from concourse.bass_utils import run_bass_kernel_spmd

B, T, C, H, D = 2, 2048, 1024, 16, 64
HPC = 4          # heads per core
G = H // HPC     # head groups (cores per batch)
N_CORES = B * G
SCALE = 1.0 / np.sqrt(D)
P = 128
QT = 512         # q tile width
TT = T // QT     # q tiles
NKB = T // P     # 128-wide k blocks
NTB = T // P     # 128-wide t blocks
NCC = C // P     # 128-deep contraction chunks
F32 = mybir.dt.float32
F32R = mybir.dt.float32r


def _tril_mask():
    p = np.arange(P)[:, None]
    f = np.arange(P)[None, :]
    return (p <= f).astype(np.float32)


# walrus in this toolchain can't encode >1 sem wait on one instruction
# ("Too many sync wait commands"); split excess waits onto preceding NoOps.
def _split_waits(nc, maxw=1):
    for f in nc.m.functions:
        for bb in f.blocks:
            out = []
            for inst in bb.instructions:
                si = getattr(inst, 'sync_info', None)
                if si is not None and si.on_wait and len(si.on_wait) > maxw:
                    waits = list(si.on_wait)
                    extra, keep = waits[:-maxw], waits[-maxw:]
                    for i in range(0, len(extra), maxw):
                        out.append(mybir.InstNoOp(
                            name=f"{inst.name}-wsplit{i}",
                            sync_info=mybir.SyncInfo(
                                on_wait=extra[i:i + maxw], on_update=[]),
                            bass_nofuse=True,
                            engine=inst.engine,
                        ))
                    inst.sync_info = mybir.SyncInfo(
                        on_wait=keep, on_update=list(si.on_update or []))
                out.append(inst)
            bb.instructions[:] = out


def build_nc(split=True):
    nc = bass.Bass()
    xT = nc.dram_tensor("xT", [C, T], F32R, kind="ExternalInput")
    wq = nc.dram_tensor("wq", [C, 256], F32R, kind="ExternalInput")
    wk = nc.dram_tensor("wk", [C, 256], F32R, kind="ExternalInput")
    wv = nc.dram_tensor("wv", [C, 260], F32R, kind="ExternalInput")
    bqk = nc.dram_tensor("bqk", [P, 4], F32, kind="ExternalInput")  # qA qB kA kB
    bv = nc.dram_tensor("bv", [P, 2], F32, kind="ExternalInput")
    scs = nc.dram_tensor("scs", [P, 2 * T], F32, kind="ExternalInput")  # sin|cos
    wout = nc.dram_tensor("wout", [256, C], F32R, kind="ExternalInput")
    y = nc.dram_tensor("y", [T, C], F32, kind="ExternalOutput")
    masks_d = nc.inline_tensor(_tril_mask(), name="cmasks")

    with tile.TileContext(nc) as tc:
        with ExitStack() as ctx:
            # ---- resident pools ----
            wpool = ctx.enter_context(tc.tile_pool(name="wts", bufs=1))
            qkpool = ctx.enter_context(tc.tile_pool(name="qk", bufs=1))
            vpool = ctx.enter_context(tc.tile_pool(name="v", bufs=1))
            otpool = ctx.enter_context(tc.tile_pool(name="ot", bufs=1))

            bqk_sb = wpool.tile([P, 4], F32, tag="bqk")
            nc.sync.dma_start(bqk_sb[:], bqk[:])
            bv_sb = wpool.tile([P, 2], F32, tag="bv")
            nc.sync.dma_start(bv_sb[:], bv[:])
            wq_sb = wpool.tile([P, NCC, 256], F32R, tag="wq")
            nc.sync.dma_start(wq_sb[:], wq.rearrange("(o p) n -> p o n", p=P))
            wk_sb = wpool.tile([P, NCC, 256], F32R, tag="wk")
            wv_sb = wpool.tile([P, NCC, 260], F32R, tag="wv")
            scs_sb = wpool.tile([P, 2 * T], F32, tag="scs")
            sin4 = scs_sb[:, 0:T]
            cos4 = scs_sb[:, T:2 * T]
            wout_sb = wpool.tile([P, 2, C], F32R, tag="wout")
            masks_sb = wpool.tile([P, P], F32R, tag="masks")
            ones_f = wpool.tile([P, 64], F32, tag="onesf")
            nc.vector.memset(ones_f[:], 1.0)
            ones_sb = wpool.tile([P, 64], F32R, tag="ones")
            nc.vector.tensor_copy(ones_sb[:], ones_f[:])

            # q/k in rotary-half layout: A = low halves of 4 heads, B = high
            qA = qkpool.tile([P, T], F32R, tag="qA")
            qB = qkpool.tile([P, T], F32R, tag="qB")
            kA = qkpool.tile([P, T], F32R, tag="kA")
            kB = qkpool.tile([P, T], F32R, tag="kB")
            qk_tiles = [qA, qB, kA, kB]
            w_of = {0: wq_sb, 1: wq_sb, 2: wk_sb, 3: wk_sb}
            col_of = {0: 0, 1: 128, 2: 0, 3: 128}

            # V tiles [t-block, 4*65] (65th col per head becomes ones)
            v_tiles = [vpool.tile([P, 260], F32R, tag=f"v{tb}", name=f"v{tb}")
                       for tb in range(NTB)]

            # O^T: heads 0,1 stacked / heads 2,3 stacked
            ot_sb = [otpool.tile([P, T], F32R, tag=f"otsb{i}", name=f"otsb{i}")
                     for i in range(2)]

            with ExitStack() as stream:
                xpool = stream.enter_context(tc.tile_pool(name="x", bufs=2))
                rtmp = stream.enter_context(tc.tile_pool(name="rtmp", bufs=4))
                ps_s = stream.enter_context(
                    tc.tile_pool(name="pss", bufs=2, space="PSUM"))
                ps_ot = stream.enter_context(
                    tc.tile_pool(name="psot", bufs=2, space="PSUM"))
                espool = stream.enter_context(tc.tile_pool(name="es", bufs=5))
                dpool = stream.enter_context(tc.tile_pool(name="dv", bufs=2))
                opool = stream.enter_context(tc.tile_pool(name="osb", bufs=2))
                qtmp = stream.enter_context(tc.tile_pool(name="qtmp", bufs=2))

                xt = {}

                def load_xt(tt):
                    for cc in range(NCC):
                        t = xpool.tile([P, QT], F32R, tag=f"x{cc}",
                                       name=f"x{cc}_{tt}")
                        nc.sync.dma_start(
                            t[:], xT[cc * P:(cc + 1) * P,
                                     tt * QT:(tt + 1) * QT])
                        xt[(cc, tt)] = t

                # q/k projection for one (jb, tt): contraction split in two
                # 4-chunk PSUM generations so a pss slot is never held long.
                def qkproj(jb, tt):
                    wsb, c0 = w_of[jb], col_of[jb]
                    dst = qk_tiles[jb][:, tt * QT:(tt + 1) * QT]
                    psa = ps_s.tile([P, 2, QT], F32, tag="pss",
                                    name="qk_a")[:, 0, :]
                    for cc in range(4):
                        nc.tensor.matmul(
                            psa, wsb[:, cc, c0:c0 + 128], xt[(cc, tt)][:],
                            start=(cc == 0), stop=(cc == 3))
                    half = qtmp.tile([P, QT], F32, tag="qh", name="qh")
                    nc.scalar.activation(
                        half[:], psa, mybir.ActivationFunctionType.Identity,
                        bias=bqk_sb[:, jb:jb + 1], scale=1.0)
                    psb = ps_s.tile([P, 2, QT], F32, tag="pss",
                                    name="qk_b")[:, 0, :]
                    for cc in range(4, NCC):
                        nc.tensor.matmul(
                            psb, wsb[:, cc, c0:c0 + 128], xt[(cc, tt)][:],
                            start=(cc == 4), stop=(cc == NCC - 1))
                    nc.vector.tensor_add(dst, psb, half[:])

                def rope(pair, tt):
                    At, Bt = qk_tiles[2 * pair], qk_tiles[2 * pair + 1]
                    s = slice(tt * QT, (tt + 1) * QT)
                    t1 = rtmp.tile([P, QT], F32R, tag="rt", name="rt1")
                    t2 = rtmp.tile([P, QT], F32R, tag="rt", name="rt2")
                    t3 = rtmp.tile([P, QT], F32R, tag="rt", name="rt3")
                    cosr = cos4[:, s].bitcast(F32R)
                    sinr = sin4[:, s].bitcast(F32R)
                    nc.gpsimd.tensor_mul(t1[:], At[:, s], cosr)
                    nc.gpsimd.tensor_mul(t2[:], Bt[:, s], sinr)
                    nc.vector.tensor_mul(t3[:], At[:, s], sinr)
                    nc.vector.tensor_sub(At[:, s], t1[:], t2[:])
                    nc.vector.tensor_mul(Bt[:, s], Bt[:, s], cosr)
                    nc.vector.tensor_add(Bt[:, s], Bt[:, s], t3[:])

                def vproj(tb):
                    ps = ps_s.tile([P, 2, QT], F32, tag="pss",
                                   name="psv")[:, 0, 0:260]
                    for cc in range(NCC):
                        nc.tensor.matmul(
                            ps, xt[(cc, tb // 4)][:, (tb % 4) * P:
                                                  (tb % 4 + 1) * P],
                            wv_sb[:, cc, :],
                            start=(cc == 0), stop=(cc == NCC - 1))
                    nc.vector.tensor_copy(v_tiles[tb][:], ps)
                    ones_cols = v_tiles[tb].rearrange(
                        "p (h e) -> p h e", e=65)[:, :, 64]
                    nc.vector.tensor_scalar_add(ones_cols, ones_cols, 1.0)

                def divides_a(tt, ot2):
                    pend = []
                    for pp in range(2):
                        otf2 = dpool.tile([65, 2, QT], F32R, tag="otf",
                                          name="otf")
                        nc.scalar.activation(
                            otf2[:], ot2[pp][:],
                            mybir.ActivationFunctionType.Copy)
                        dn = dpool.tile([P, 8], F32, tag="dn", name="dn")
                        nc.sync.dma_start(
                            dn[:], otf2[64:65, :, :].rearrange(
                                "a b c -> a (b c)").bitcast(F32))
                        nc.vector.reciprocal(dn[:], dn[:])
                        rr = dpool.tile([1, 2, QT], F32R, tag="rr", name="rr")
                        nc.sync.dma_start(
                            rr[0:1, :, :].rearrange("a b c -> a (b c)"),
                            dn[:].bitcast(F32R))
                        pend.append((otf2, rr))
                    return pend

                def divides_b(tt, pend):
                    for pp in range(2):
                        otf2, rr = pend[pp]
                        for j in range(2):
                            rb = ps_s.tile([P, 2, QT], F32, tag="pss",
                                           name="rb")[0:64, 0, :]
                            nc.tensor.matmul(
                                rb, ones_sb[0:1, 0:64],
                                rr[0:1, j, :], start=True, stop=True)
                            rbs = dpool.tile([64, QT], F32R, tag="rbs",
                                             name="rbs")
                            nc.vector.tensor_copy(rbs[:], rb.bitcast(F32R))
                            dst = ot_sb[pp][64 * j:64 * j + 64,
                                            tt * QT:(tt + 1) * QT]
                            nc.vector.tensor_mul(dst, otf2[0:64, j, :], rbs[:])
                            nc.vector.tensor_scalar_add(
                                dst, dst, bv_sb[64 * j:64 * j + 64, pp:pp + 1])

                def outproj(tb):
                    o_sb = opool.tile([P, C], F32, tag="osb", name="osb")
                    for nt in range(2):
                        ps = ps_s.tile([P, 2, QT], F32, tag="pss",
                                       name="pso")[:, 0, :]
                        for rc in range(2):
                            nc.tensor.matmul(
                                ps, ot_sb[rc][:, tb * P:(tb + 1) * P],
                                wout_sb[:, rc, nt * 512:(nt + 1) * 512],
                                start=(rc == 0), stop=(rc == 1))
                        nc.vector.tensor_copy(
                            o_sb[:, nt * 512:(nt + 1) * 512], ps)
                    nc.sync.dma_start(y[tb * P:(tb + 1) * P, :], o_sb[:])

                # ---- prologue: tile 0's inputs and projections ----
                load_xt(0)
                nc.sync.dma_start(wk_sb[:], wk.rearrange("(o p) n -> p o n", p=P))
                nc.sync.dma_start(scs_sb[:], scs[:])
                nc.sync.dma_start(wv_sb[:], wv.rearrange("(o p) n -> p o n", p=P))
                nc.sync.dma_start(masks_sb[:], masks_d[:].bitcast(F32R))
                nc.sync.dma_start(wout_sb[:],
                                  wout.rearrange("(o p) n -> p o n", p=P))
                for jb in range(4):
                    qkproj(jb, 0)
                rope(0, 0)
                rope(1, 0)
                for tb in range(4):
                    vproj(tb)

                # ---- streaming attention with injected work ----
                queue = []  # closures of next-tile + prev-tile work
                prev = None
                for tt in range(TT):
                    nk = 4 * tt + 4
                    if tt + 1 < TT:
                        ntt = tt + 1
                        load_xt(ntt)
                        queue += [lambda jb=jb, t=ntt: qkproj(jb, t)
                                  for jb in range(4)]
                        queue += [lambda t=ntt: rope(0, t),
                                  lambda t=ntt: rope(1, t)]
                        queue += [lambda tb=tb: vproj(tb)
                                  for tb in range(4 * ntt, 4 * ntt + 4)]
                    ot2 = [ps_ot.tile([65, 2, QT], F32, tag="psot",
                                      name=f"psot{pp}") for pp in range(2)]
                    es_prev = [None, None]
                    off_prev = [0, 0]
                    for kblk in range(nk):
                        off = max(0, (kblk - 4 * tt)) * P
                        ks = slice(kblk * P, (kblk + 1) * P)
                        qs = slice(tt * QT + off, (tt + 1) * QT)
                        for pp in range(2):
                            s2 = ps_s.tile([P, 2, QT], F32, tag="pss",
                                           name="pss")
                            for j in range(2):
                                h = 2 * pp + j
                                hp = slice(32 * h, 32 * h + 32)
                                nc.tensor.matmul(
                                    s2[:, j, off:], kA[hp, ks], qA[hp, qs],
                                    start=True, stop=False,
                                    tile_position=(32 * h, 0))
                                nc.tensor.matmul(
                                    s2[:, j, off:], kB[hp, ks], qB[hp, qs],
                                    start=False, stop=True,
                                    tile_position=(32 * h, 0))
                            es2 = espool.tile([P, 2, QT], F32R, tag="es",
                                              name="es")
                            nc.scalar.activation(
                                es2[:, :, off:], s2[:, :, off:],
                                mybir.ActivationFunctionType.Exp, scale=SCALE)
                            if kblk >= 4 * tt:
                                eng = nc.vector if pp == 0 else nc.gpsimd
                                eng.tensor_mul(
                                    es2[:, :, off:off + P],
                                    es2[:, :, off:off + P],
                                    masks_sb[:, None, :].to_broadcast(
                                        (P, 2, P)))
                            if kblk > 0:
                                for j in range(2):
                                    h = 2 * pp + j
                                    nc.tensor.matmul(
                                        ot2[pp][:, j, off_prev[pp]:],
                                        v_tiles[kblk - 1][:, 65 * h:65 * h + 65],
                                        es_prev[pp][:, j, off_prev[pp]:],
                                        start=(kblk == 1), stop=False)
                            es_prev[pp], off_prev[pp] = es2, off
                        # injected pipeline work
                        if prev is not None:
                            ptt, pend = prev
                            if kblk == 2:
                                divides_b(ptt, pend)
                            elif 3 <= kblk <= 6:
                                outproj(4 * ptt + kblk - 3)
                                if kblk == 6:
                                    prev = None
                        rounds_left = nk - 1 - kblk
                        if queue:
                            npop = max(1, -(-len(queue) // max(1, rounds_left)))                                 if rounds_left > 0 else len(queue)
                            for _ in range(min(npop, len(queue))):
                                queue.pop(0)()
                    for pp in range(2):
                        for j in range(2):
                            h = 2 * pp + j
                            nc.tensor.matmul(
                                ot2[pp][:, j, off_prev[pp]:],
                                v_tiles[nk - 1][:, 65 * h:65 * h + 65],
                                es_prev[pp][:, j, off_prev[pp]:],
                                start=(nk == 1), stop=True)
                    prev = (tt, divides_a(tt, ot2))
                # tail
                ptt, pend = prev
                divides_b(ptt, pend)
                for tb in range(4 * ptt, 4 * ptt + 4):
                    outproj(tb)

    if split:
        _split_waits(nc)
    return nc


def make_in_maps(x, rope_cache, Wqkv, bqkv, Wout, bout):
    """Host-side shard prep. Returns list of 8 in_maps (core = 4*b + g)."""
    x = np.asarray(x, np.float32)
    rope_cache = np.asarray(rope_cache, np.float32)
    Wqkv = np.asarray(Wqkv, np.float32)
    bqkv = np.asarray(bqkv, np.float32)
    Wout = np.asarray(Wout, np.float32)

    # rotary-half permutation within a head: [evens, odds]
    perm = np.concatenate([np.arange(0, D, 2), np.arange(1, D, 2)])
    sin = rope_cache[:, 0::2].T.copy()   # [32, T]
    cos = rope_cache[:, 1::2].T.copy()
    scs = np.concatenate([np.tile(sin, (4, 1)), np.tile(cos, (4, 1))],
                         axis=1).astype(np.float32)  # [128, 2T]

    xT = [np.ascontiguousarray(x[b].T) for b in range(B)]

    in_maps = []
    for core in range(N_CORES):
        b, g = divmod(core, G)
        heads = range(HPC * g, HPC * g + HPC)
        # A-block: low halves (even dims) of the 4 heads; B-block: high halves
        qcols, kcols, vcols = [], [], []
        for part in range(2):  # lo, hi
            for h in heads:
                dd = h * D + perm[part * 32:(part + 1) * 32]
                qcols.extend(0 * C + dd)
                kcols.extend(1 * C + dd)
        for h in heads:
            vcols.extend(2 * C + h * D + np.arange(D))
        qcols = np.asarray(qcols)
        kcols = np.asarray(kcols)
        vcols = np.asarray(vcols)
        wq_c = np.ascontiguousarray(Wqkv[:, qcols])
        wk_c = np.ascontiguousarray(Wqkv[:, kcols])
        wv_c = np.zeros((C, 260), np.float32)
        vv = Wqkv[:, vcols]
        for h in range(HPC):
            wv_c[:, 65 * h:65 * h + 64] = vv[:, 64 * h:64 * h + 64]
        bqk_c = np.stack([bqkv[qcols[:128]], bqkv[qcols[128:]],
                          bqkv[kcols[:128]], bqkv[kcols[128:]]], axis=1)
        bv_c = bqkv[vcols].reshape(2, 128).T
        rows = np.arange(HPC * g * D, (HPC * g + HPC) * D)
        wout_c = np.ascontiguousarray(Wout[rows, :])
        in_maps.append({
            "xT": xT[b], "wq": wq_c, "wk": wk_c,
            "wv": np.ascontiguousarray(wv_c),
            "bqk": np.ascontiguousarray(bqk_c.astype(np.float32)),
            "bv": np.ascontiguousarray(bv_c.astype(np.float32)),
            "scs": scs, "wout": wout_c,
        })
    return in_maps


_NC_CACHE = None


def _get_nc():
    global _NC_CACHE
    if _NC_CACHE is None:
        _NC_CACHE = build_nc()
    return _NC_CACHE


def run(inputs, trace=False):
    nc = _get_nc()
    in_maps = make_in_maps(**inputs)
    res = run_bass_kernel_spmd(nc, in_maps, list(range(N_CORES)), trace=trace)
    bout = np.asarray(inputs["bout"], np.float32)
    out = np.zeros((B, T, C), np.float32)
    for core in range(N_CORES):
        out[core // G] += res.results[core]["y"]
    out += bout[None, None, :]
    return out, res


def kernel(**inputs):
    out, _ = run(inputs)
    return out
